# revision 31
# baseline (speedup 1.0000x reference)
"""Trainium2 Bass kernel for nn_BaseModel_32255204393001 (v2).

Sharding (8 cores): batch 256 -> 2 groups of 128 (cores 0-3 / 4-7). Within a
group, 4 lanes shard GRU gates (768/lane), attention heads (1/lane), spline
joints (4/lane, lane3 dup-padded). Per-step hidden AllGather within the group.

v2 changes vs baseline:
- x^T layouts precomputed on host (kills device transpose phase)
- attention in bf16 with fused freq_w@attn_in_w, mean-attention u-trick
  (never materializes V), issued as ~110 small stages interleaved into the
  encode scan to fill PE idle time during AllGathers
- bias rows folded into lhsT ones-rows where possible
- AllGather staging spread across engine queues, single-DMA unpack
- debug outputs dropped
"""

import numpy as np
import ml_dtypes

import concourse.bacc as bacc
import concourse.mybir as mybir
import concourse.tile as tile
from concourse.masks import make_identity

F32 = mybir.dt.float32
F16 = mybir.dt.float16
BF16 = mybir.dt.bfloat16
AF = mybir.ActivationFunctionType
ALU = mybir.AluOpType

B, T, PRED, J, H, D = 256, 120, 24, 15, 1024, 135
HEADS = 4
FQ = T // 2 + 1          # 61 freq bins
BC = 128                 # batch per group
GSL = 768                # gate slice per lane (r|z|n 256 each)
JC = 4                   # joints per lane (lane3: 3 real + 1 dup)
CB = 32                  # attention batch chunk
GROUPS = [[0, 1, 2, 3], [4, 5, 6, 7]]

T_STEPS = T
PRED_STEPS = PRED


def _bf(x):
    return np.asarray(x, dtype=ml_dtypes.bfloat16)


def build_module(t_steps=T_STEPS, pred_steps=PRED_STEPS, debug=False):
    nc = bacc.Bacc("TRN2", target_bir_lowering=False, debug=False, num_devices=8)

    def din(name, shape, dt=BF16):
        return nc.dram_tensor(name, shape, dt, kind="ExternalInput")

    # GRU / rollout inputs
    XThi_d = din("XThi", [128, t_steps * 128])
    XTlo_d = din("XTlo", [8, t_steps * 128])      # row 7 = ones
    x0Th = din("x0Th", [128, BC])
    x0Tl = din("x0Tl", [8, BC])                   # row 7 = ones
    prev6d0 = din("prev6d0_s", [BC, JC * 6], F32)
    wih0T = din("wih0T_s", [128, GSL])
    wih0lo = din("wih0lo_s", [8, GSL])            # row 7 = brz0|bni0
    whh0T = din("whh0T_s", [H, GSL])
    wih1T = din("wih1T_s", [H, GSL])
    whh1T = din("whh1T_s", [H, GSL])
    bnh0 = din("bnh0", [1, 256])
    brz1 = din("brz1", [1, 512]); bni1 = din("bni1", [1, 256]); bnh1 = din("bnh1", [1, 256])
    pre_wT = din("pre_wT", [H, H])
    pre_b = din("pre_b_row", [1, H])
    spl1T = din("spl1T", [H, JC * 128])
    spl1b = din("spl1b", [1, JC * 128])
    spl2 = din("spl2_s", [128, JC * 6])
    spl2b = din("spl2b", [1, JC * 6])
    # attention inputs (score path fp32, post-softmax bf16)
    posesT = din("posesT16", [T, BC * D], F16)
    ct_b = din("ct16", [T, FQ], F16)
    ctT_b = din("ctT_b", [FQ, T])
    wfq_hi = din("wfq_hi", [128, GSL], F16)       # cols q(256)|k(256)|v(256)
    wfq_lo = din("wfq_lo", [8, GSL], F16)         # row 7 = fused qkv bias
    wv_hi = din("wv_hi", [128, 256])              # bf16 copy of v cols
    wv_lo = din("wv_lo", [8, 256])
    wout_h = din("wout_h", [256, H])
    aob4 = din("aob4", [1, H])

    out6d = nc.dram_tensor("out6d", [PRED, BC, JC * 6], F32, kind="ExternalOutput")
    if debug:
        dbg_ctx = nc.dram_tensor("dbg_ctx", [BC, H], F32, kind="ExternalOutput")
        dbg_h0 = nc.dram_tensor("dbg_h0", [BC, 256], F32, kind="ExternalOutput")
        dbg_h1 = nc.dram_tensor("dbg_h1", [BC, 256], F32, kind="ExternalOutput")
        dbg_h0T = nc.dram_tensor("dbg_h0T", [128, H], F32, kind="ExternalOutput")
        dbg_O = nc.dram_tensor("dbg_O", [128, BC], F32, kind="ExternalOutput")
        dbg_A = nc.dram_tensor("dbg_A", [FQ, CB], F32, kind="ExternalOutput")

    with tile.TileContext(nc) as tc:
        # ---------------- persistent pool: weights + state ----------------
        wp_cm = tc.tile_pool(name="wp", bufs=1)
        wp = wp_cm.__enter__()
        ident = wp.tile([128, 128], BF16)
        make_identity(nc, ident)
        ones_col = wp.tile([1, 128], BF16)
        nc.vector.memset(ones_col[:], 1.0)

        def load(t_dram, dt=BF16):
            tl = wp.tile(list(t_dram.shape), dt, tag=t_dram.name)
            nc.sync.dma_start(out=tl[:], in_=t_dram[:])
            return tl

        XThi = load(XThi_d)
        XTlo = load(XTlo_d)
        wih0hi = load(wih0T)
        wih0lo_sb = load(wih0lo)
        whh0_k = [wp.tile([128, GSL], BF16, tag=f"whh0_{k}", name=f"whh0_{k}") for k in range(8)]
        wih1_k = [wp.tile([128, GSL], BF16, tag=f"wih1_{k}", name=f"wih1_{k}") for k in range(8)]
        whh1_k = [wp.tile([128, GSL], BF16, tag=f"whh1_{k}", name=f"whh1_{k}") for k in range(8)]
        for k in range(8):
            nc.sync.dma_start(out=whh0_k[k][:], in_=whh0T[k * 128:(k + 1) * 128, :])
            nc.sync.dma_start(out=wih1_k[k][:], in_=wih1T[k * 128:(k + 1) * 128, :])
            nc.sync.dma_start(out=whh1_k[k][:], in_=whh1T[k * 128:(k + 1) * 128, :])
        bnh0_sb = load(bnh0)
        brz1_sb = load(brz1); bni1_sb = load(bni1); bnh1_sb = load(bnh1)
        x0Th_sb = load(x0Th)
        x0Tl_sb = load(x0Tl)
        p6d = wp.tile([BC, JC * 6], F32, tag="p6d")
        nc.sync.dma_start(out=p6d[:], in_=prev6d0[:])
        # attention weights
        ct_sb = load(ct_b, dt=F16)
        ctT_sb = load(ctT_b)
        wfq_hi_sb = load(wfq_hi, dt=F16)
        wfq_lo_sb = load(wfq_lo, dt=F16)
        wv_hi_sb = load(wv_hi)
        wv_lo_sb = load(wv_lo)
        wout0 = wp.tile([128, H], BF16, tag="wout0")
        nc.sync.dma_start(out=wout0[:], in_=wout_h[0:128, :])
        wout1 = wp.tile([128, H], BF16, tag="wout1")
        nc.sync.dma_start(out=wout1[:], in_=wout_h[128:256, :])
        aob_sb = load(aob4)

        # state
        h0 = wp.tile([BC, 256], F32, tag="h0"); nc.vector.memset(h0[:], 0.0)
        h1 = wp.tile([BC, 256], F32, tag="h1"); nc.vector.memset(h1[:], 0.0)
        h0T = wp.tile([128, H], BF16, tag="h0T"); nc.vector.memset(h0T[:], 0.0)
        h1T = wp.tile([128, H], BF16, tag="h1T"); nc.vector.memset(h1T[:], 0.0)
        ctx_sb = wp.tile([BC, H], F32, tag="ctx_sb")

        # ---------------- attention stage machinery ----------------
        ap_cm = tc.tile_pool(name="ap", bufs=1)
        ap = ap_cm.__enter__()
        ap_ps_cm = tc.tile_pool(name="apps", bufs=1, space="PSUM")
        ap_ps = ap_ps_cm.__enter__()
        O_sb0 = ap.tile([128, BC], BF16, tag="O_sb0")
        O_sb1 = ap.tile([128, BC], BF16, tag="O_sb1")
        XCb = ap.tile([T, CB * D], F16, tag="XCb")
        fhi = ap.tile([128, FQ * CB], F16, tag="fhi")
        flo = ap.tile([8, FQ * CB], F16, tag="flo")
        qk_sb = [ap.tile([128, FQ * CB], F16, tag=f"qk{m}", name=f"qk{m}") for m in range(4)]
        S_sb = ap.tile([FQ, FQ * CB], F32, tag="S_sb")
        e_sb = ap.tile([FQ, FQ * CB], BF16, tag="e_sb")
        mx_t = ap.tile([FQ, CB], F32, tag="mx")
        sm_t = ap.tile([FQ, CB], F32, tag="sm")
        rs_t = ap.tile([FQ, CB], F32, tag="rs")
        rsb_t = ap.tile([FQ, CB], BF16, tag="rsb")
        A_b = ap.tile([FQ, CB], BF16, tag="A_b")
        Wt_w = ap.tile([T, CB], F16, tag="Wt_w")
        U_hi = ap.tile([128, CB], BF16, tag="U_hi")
        U_lo = ap.tile([8, CB], BF16, tag="U_lo")
        arin_b = ap.tile([BC, H], BF16, tag="arin_b")
        ctx_bf = ap.tile([BC, H], BF16, tag="ctx_bf")

        stages = []

        def add_stage(fn):
            stages.append(fn)

        for ci in range(BC // CB):
            def st_load(ci=ci):
                nc.sync.dma_start(out=XCb[:], in_=posesT[:, ci * CB * D:(ci + 1) * CB * D])
                nc.vector.memset(flo[:], 1.0)
                nc.vector.memset(U_lo[:], 1.0)
            add_stage(st_load)
            for g in range(4):   # 8 bi per group
                def st_pdd(g=g):
                    pdd = ap_ps.tile([128, 8 * FQ], F32, tag="big")
                    pdl = ap_ps.tile([8, 8 * FQ], F32, tag="sm1")
                    for i8 in range(8):
                        bi = g * 8 + i8
                        sl = slice(i8 * FQ, (i8 + 1) * FQ)
                        nc.tensor.matmul(pdd[:, sl], XCb[:, bi * D: bi * D + 128], ct_sb[:], start=True, stop=True)
                        nc.tensor.matmul(pdl[0:7, sl], XCb[:, bi * D + 128: (bi + 1) * D], ct_sb[:], start=True, stop=True)
                    dsl = slice(g * 8 * FQ, (g + 1) * 8 * FQ)
                    nc.vector.tensor_copy(fhi[:, dsl], pdd[:])
                    nc.vector.tensor_copy(flo[0:7, dsl], pdl[0:7, :])
                add_stage(st_pdd)
            for m in range(4):   # qkv proj: q0 q1 k0 k1
                def st_qk(m=m):
                    NCB = FQ * CB
                    for s in range(4):
                        c0 = s * 488
                        c1 = min(c0 + 488, NCB)
                        pq = ap_ps.tile([128, 488], F32, tag="big")
                        nc.tensor.matmul(pq[:, 0:c1 - c0], wfq_hi_sb[:, m * 128:(m + 1) * 128], fhi[:, c0:c1], start=True, stop=False)
                        nc.tensor.matmul(pq[:, 0:c1 - c0], wfq_lo_sb[:, m * 128:(m + 1) * 128], flo[:, c0:c1], start=False, stop=True)
                        nc.vector.tensor_copy(qk_sb[m][:, c0:c1], pq[:, 0:c1 - c0])
                add_stage(st_qk)
            for g in range(4):
                def st_S(g=g):
                    pS = ap_ps.tile([FQ, 8 * FQ], F32, tag="big")
                    for i8 in range(8):
                        bi = g * 8 + i8
                        sl = slice(bi * FQ, (bi + 1) * FQ)
                        psl = slice(i8 * FQ, (i8 + 1) * FQ)
                        nc.tensor.matmul(pS[:, psl], qk_sb[0][:, sl], qk_sb[2][:, sl], start=True, stop=False)
                        nc.tensor.matmul(pS[:, psl], qk_sb[1][:, sl], qk_sb[3][:, sl], start=False, stop=True)
                    nc.vector.tensor_copy(S_sb[:, g * 8 * FQ:(g + 1) * 8 * FQ], pS[:])
                add_stage(st_S)

                def st_sm(g=g):
                    c0 = g * 8 * FQ
                    S3 = S_sb[:, c0:c0 + 8 * FQ].rearrange("p (b k) -> p b k", k=FQ)
                    gb = slice(g * 8, (g + 1) * 8)
                    nc.vector.reduce_max(mx_t[:, gb, None], S3, axis=mybir.AxisListType.X)
                    nc.vector.tensor_tensor(out=S3, in0=S3,
                                            in1=mx_t[:, gb, None].broadcast_to([FQ, 8, FQ]),
                                            op=ALU.subtract)
                    nc.scalar.activation(e_sb[:, c0:c0 + 8 * FQ], S_sb[:, c0:c0 + 8 * FQ], AF.Exp, scale=1.0 / 16.0)
                    e3 = e_sb[:, c0:c0 + 8 * FQ].rearrange("p (b k) -> p b k", k=FQ)
                    nc.vector.reduce_sum(sm_t[:, gb, None], e3, axis=mybir.AxisListType.X)
                    nc.vector.reciprocal(rs_t[:, gb], sm_t[:, gb])
                    nc.vector.tensor_scalar_mul(rsb_t[:, gb], rs_t[:, gb], 1.0 / FQ)
                add_stage(st_sm)
            def st_A(ci=ci):
                pA = ap_ps.tile([FQ, CB], F32, tag="sm1")
                for bi in range(CB):
                    nc.tensor.matmul(pA[:, bi:bi + 1], e_sb[:, bi * FQ:(bi + 1) * FQ],
                                     rsb_t[:, bi:bi + 1], start=True, stop=True)
                nc.vector.tensor_copy(A_b[:], pA[:])
                pW = ap_ps.tile([T, CB], F32, tag="sm2")
                nc.tensor.matmul(pW[:], ctT_sb[:], A_b[:], start=True, stop=True)
                nc.vector.tensor_copy(Wt_w[:], pW[:])
            add_stage(st_A)
            for g in range(4):
                def st_u(g=g, ci=ci):
                    pUh = ap_ps.tile([128, 8], F32, tag="sm1")
                    pUl = ap_ps.tile([8, 8], F32, tag="sm2")
                    for i8 in range(8):
                        bi = g * 8 + i8
                        nc.tensor.matmul(pUh[:, i8:i8 + 1], XCb[:, bi * D: bi * D + 128],
                                         Wt_w[:, bi:bi + 1], start=True, stop=True)
                        nc.tensor.matmul(pUl[0:7, i8:i8 + 1], XCb[:, bi * D + 128:(bi + 1) * D],
                                         Wt_w[:, bi:bi + 1], start=True, stop=True)
                    gb = slice(g * 8, (g + 1) * 8)
                    nc.vector.tensor_copy(U_hi[:, gb], pUh[:])
                    nc.vector.tensor_copy(U_lo[0:7, gb], pUl[0:7, :])
                add_stage(st_u)
            def st_O(ci=ci):
                pO0 = ap_ps.tile([128, CB], F32, tag="sm1")
                pO1 = ap_ps.tile([128, CB], F32, tag="sm2")
                nc.tensor.matmul(pO0[:], wv_hi_sb[:, 0:128], U_hi[:], start=True, stop=False)
                nc.tensor.matmul(pO0[:], wv_lo_sb[:, 0:128], U_lo[:], start=False, stop=True)
                nc.tensor.matmul(pO1[:], wv_hi_sb[:, 128:256], U_hi[:], start=True, stop=False)
                nc.tensor.matmul(pO1[:], wv_lo_sb[:, 128:256], U_lo[:], start=False, stop=True)
                nc.vector.tensor_copy(O_sb0[:, ci * CB:(ci + 1) * CB], pO0[:])
                nc.vector.tensor_copy(O_sb1[:, ci * CB:(ci + 1) * CB], pO1[:])
            add_stage(st_O)

        ar_dr_cm = tc.tile_pool(name="ardr", bufs=1, space="DRAM")
        ar_dr = ar_dr_cm.__enter__()

        def st_arin():
            for s in range(2):
                sl = slice(s * 512, (s + 1) * 512)
                pc = ap_ps.tile([BC, 512], F32, tag="big")
                nc.tensor.matmul(pc[:], O_sb0[:], wout0[:, sl], start=True, stop=False)
                nc.tensor.matmul(pc[:], O_sb1[:], wout1[:, sl], start=False, stop=False)
                nc.tensor.matmul(pc[:], ones_col[:, 0:BC], aob_sb[:, sl], start=False, stop=True)
                nc.vector.tensor_copy(arin_b[:, sl], pc[:])
            ar_i = ar_dr.tile([BC, H], BF16)
            ar_o = ar_dr.tile([BC, H], BF16)
            nc.sync.dma_start(out=ar_i[:], in_=arin_b[:])
            nc.gpsimd.collective_compute("AllReduce", ALU.add, ins=[ar_i.opt()],
                                         outs=[ar_o.opt()], replica_groups=GROUPS)
            nc.sync.dma_start(out=ctx_bf[:], in_=ar_o[:])
            nc.vector.tensor_copy(ctx_sb[:], ctx_bf[:])
        add_stage(st_arin)

        # ---------------- GRU cell emitters ----------------
        def emit_cell_whh(ps, whh, bnh, hT_own):
            psA = ps.tile([BC, 512], F32, tag="A")
            psB = ps.tile([BC, 256], F32, tag="B")
            psC = ps.tile([BC, 256], F32, tag="C")
            for k in range(8):
                nc.tensor.matmul(psA[:], hT_own[:, k * 128:(k + 1) * 128], whh[k][:, 0:512],
                                 start=(k == 0), stop=False)
            for k in range(8):
                nc.tensor.matmul(psB[:], hT_own[:, k * 128:(k + 1) * 128], whh[k][:, 512:768],
                                 start=(k == 0), stop=False)
            nc.tensor.matmul(psB[:], ones_col[:, 0:BC], bnh[:], start=False, stop=True)
            return psA, psB, psC

        def emit_cell(ps, trps, sb, dr, layer, t, whh, brz_bias, bnh, bni_bias,
                      h_own, hT_own, ih_fn, pre_mm=None, ih_first=False):
            """ih_fn(psA, psC) emits the input-projection matmuls (incl. fused
            bias rows where available). brz_bias/bni_bias are None when folded."""
            if pre_mm is not None:
                psA, psB, psC = pre_mm
                ih_fn(psA, psC)
            elif ih_first:
                psA = ps.tile([BC, 512], F32, tag="A")
                psB = ps.tile([BC, 256], F32, tag="B")
                psC = ps.tile([BC, 256], F32, tag="C")
                ih_fn(psA, psC)
                for k in range(8):
                    nc.tensor.matmul(psA[:], hT_own[:, k * 128:(k + 1) * 128], whh[k][:, 0:512],
                                     start=False, stop=(k == 7 and brz_bias is None))
                for k in range(8):
                    nc.tensor.matmul(psB[:], hT_own[:, k * 128:(k + 1) * 128], whh[k][:, 512:768],
                                     start=(k == 0), stop=False)
                nc.tensor.matmul(psB[:], ones_col[:, 0:BC], bnh[:], start=False, stop=True)
            else:
                psA, psB, psC = emit_cell_whh(ps, whh, bnh, hT_own)
                ih_fn(psA, psC)
            if brz_bias is not None:
                nc.tensor.matmul(psA[:], ones_col[:, 0:BC], brz_bias[:], start=False, stop=True)
            if bni_bias is not None:
                nc.tensor.matmul(psC[:], ones_col[:, 0:BC], bni_bias[:], start=False, stop=True)
            rz = sb.tile([BC, 512], F32, tag="rz")
            nc.scalar.activation(rz[:], psA[:], AF.Sigmoid)
            t1 = sb.tile([BC, 256], F32, tag="t1")
            nc.vector.tensor_tensor(out=t1[:], in0=psB[:], in1=rz[:, 0:256], op=ALU.mult)
            nc.vector.tensor_tensor(out=t1[:], in0=t1[:], in1=psC[:], op=ALU.add)
            nn_ = sb.tile([BC, 256], F32, tag="nn")
            nc.scalar.activation(nn_[:], t1[:], AF.Tanh)
            nc.vector.tensor_tensor(out=t1[:], in0=h_own[:], in1=nn_[:], op=ALU.subtract)
            nc.vector.tensor_tensor(out=t1[:], in0=t1[:], in1=rz[:, 256:512], op=ALU.mult)
            nc.vector.tensor_tensor(out=h_own[:], in0=nn_[:], in1=t1[:], op=ALU.add)
            hb = sb.tile([BC, 256], BF16, tag=f"hb{layer}")
            nc.vector.tensor_copy(hb[:], h_own[:])
            agin = sb.tile([128, 256], BF16, tag=f"agin{layer}")
            for half in range(2):
                pT = trps.tile([128, 128], BF16, tag="Tr")
                nc.tensor.transpose(pT[:], hb[:, half * 128:(half + 1) * 128], ident[:])
                nc.vector.tensor_copy(agin[:, half * 128:(half + 1) * 128], pT[:])
            return agin

        def emit_ag(dr, layer, agin, hT_own, stage_eng, unpack_eng):
            ag_i = dr.tile([128, 256], BF16, tag=f"agi{layer}")
            ag_o = dr.tile([512, 256], BF16, tag=f"ago{layer}")
            stage_eng.dma_start(out=ag_i[:], in_=agin[:])
            nc.gpsimd.collective_compute("AllGather", ALU.bypass, ins=[ag_i.opt()],
                                         outs=[ag_o.opt()], replica_groups=GROUPS)
            unpack_eng.dma_start(
                out=hT_own[:].rearrange("p (r c) -> p r c", r=4),
                in_=ag_o[:].rearrange("(r p) c -> p r c", r=4))

        def ih0_fn(lhsT_hi, lhsT_lo, first=False):
            def fn(psA, psC):
                nc.tensor.matmul(psA[:], lhsT_hi, wih0hi[:, 0:512], start=first, stop=False)
                nc.tensor.matmul(psA[:], lhsT_lo, wih0lo_sb[:, 0:512], start=False, stop=not first)
                nc.tensor.matmul(psC[:], lhsT_hi, wih0hi[:, 512:768], start=True, stop=False)
                nc.tensor.matmul(psC[:], lhsT_lo, wih0lo_sb[:, 512:768], start=False, stop=True)
            return fn

        def ih1_fn(psA, psC):
            for k in range(8):
                nc.tensor.matmul(psA[:], h0T[:, k * 128:(k + 1) * 128], wih1_k[k][:, 0:512],
                                 start=False, stop=False)
            for k in range(8):
                nc.tensor.matmul(psC[:], h0T[:, k * 128:(k + 1) * 128], wih1_k[k][:, 512:768],
                                 start=(k == 0), stop=False)

        # ---------------- encode scan (attention interleaved) ----------------
        enc_scope = nc.named_scope("encode"); enc_scope.__enter__()
        si = 0
        with tc.tile_pool(name="pc_sb", bufs=1) as pc_sb, \
             tc.tile_pool(name="pc_ps", bufs=1, space="PSUM") as pc_ps, \
             tc.tile_pool(name="pc_tr", bufs=2, space="PSUM") as pc_tr, \
             tc.tile_pool(name="pc_dr", bufs=3, space="DRAM") as pc_dr:
            for t in range(t_steps):
                agin0 = emit_cell(pc_ps, pc_tr, pc_sb, pc_dr, 0, t,
                                  whh0_k, None, bnh0_sb, None, h0, h0T,
                                  ih0_fn(XThi[:, t * 128:(t + 1) * 128],
                                         XTlo[:, t * 128:(t + 1) * 128], first=True),
                                  ih_first=True)
                emit_ag(pc_dr, 0, agin0, h0T, nc.sync, nc.sync)
                pre1 = emit_cell_whh(pc_ps, whh1_k, bnh1_sb, h1T)
                if si < len(stages):
                    stages[si](); si += 1
                agin1 = emit_cell(pc_ps, pc_tr, pc_sb, pc_dr, 1, t,
                                  whh1_k, brz1_sb, bnh1_sb, bni1_sb, h1, h1T, ih1_fn,
                                  pre_mm=pre1)
                emit_ag(pc_dr, 1, agin1, h1T, nc.sync, nc.gpsimd)
            while si < len(stages):
                stages[si](); si += 1
            if debug:
                dh0 = pc_sb.tile([BC, 256], F32, tag="dh0")
                nc.vector.tensor_copy(dh0[:], h0[:])
                nc.sync.dma_start(out=dbg_h0[:], in_=dh0[:])
                dh1 = pc_sb.tile([BC, 256], F32, tag="dh1")
                nc.vector.tensor_copy(dh1[:], h1[:])
                nc.sync.dma_start(out=dbg_h1[:], in_=dh1[:])
                dhT = pc_sb.tile([128, H], F32, tag="dhT")
                nc.vector.tensor_copy(dhT[:], h0T[:])
                nc.sync.dma_start(out=dbg_h0T[:], in_=dhT[:])
                nc.sync.dma_start(out=dbg_ctx[:], in_=ctx_sb[:])
                dO = pc_sb.tile([128, BC], F32, tag="dO")
                nc.vector.tensor_copy(dO[:], O_sb0[:])
                nc.sync.dma_start(out=dbg_O[:], in_=dO[:])
                dA = pc_sb.tile([FQ, CB], F32, tag="dA")
                nc.vector.tensor_copy(dA[:], A_b[:])
                nc.sync.dma_start(out=dbg_A[:], in_=dA[:])
        enc_scope.__exit__(None, None, None)
        ap_ps_cm.__exit__(None, None, None)
        ap_cm.__exit__(None, None, None)
        ar_dr_cm.__exit__(None, None, None)

        # ---------------- rollout-only weights ----------------
        wp2_cm = tc.tile_pool(name="wp2", bufs=1)
        wp2 = wp2_cm.__enter__()
        pre_k = [wp2.tile([128, H], BF16, tag=f"pre_{k}", name=f"pre_{k}") for k in range(8)]
        for k in range(8):
            nc.sync.dma_start(out=pre_k[k][:], in_=pre_wT[k * 128:(k + 1) * 128, :])
        pre_b_sb = wp2.tile([1, H], BF16, tag="pre_b_sb")
        nc.sync.dma_start(out=pre_b_sb[:], in_=pre_b[:])
        spl1_k = [wp2.tile([128, JC * 128], BF16, tag=f"spl1_{k}", name=f"spl1_{k}") for k in range(8)]
        for k in range(8):
            nc.sync.dma_start(out=spl1_k[k][:], in_=spl1T[k * 128:(k + 1) * 128, :])
        spl1b_sb = wp2.tile([1, JC * 128], BF16, tag="spl1b_sb")
        nc.sync.dma_start(out=spl1b_sb[:], in_=spl1b[:])
        spl2_sb = wp2.tile([128, JC * 6], BF16, tag="spl2_sb")
        nc.sync.dma_start(out=spl2_sb[:], in_=spl2[:])
        spl2b_sb = wp2.tile([1, JC * 6], BF16, tag="spl2b_sb")
        nc.sync.dma_start(out=spl2b_sb[:], in_=spl2b[:])
        xnew = wp2.tile([BC, 136], BF16, tag="xnew")
        nc.vector.memset(xnew[:, 135:136], 1.0)
        xTh = wp2.tile([128, BC], BF16, tag="xTh")
        xTl = wp2.tile([8, BC], BF16, tag="xTl")

        # ---------------- rollout ----------------
        roll_scope = nc.named_scope("rollout"); roll_scope.__enter__()
        with tc.tile_pool(name="pd_sb", bufs=2) as pd_sb, \
             tc.tile_pool(name="pd_ps", bufs=1, space="PSUM") as pd_ps, \
             tc.tile_pool(name="pd_tr", bufs=1, space="PSUM") as pd_tr, \
             tc.tile_pool(name="pd_ps2", bufs=1, space="PSUM") as pd_ps2, \
             tc.tile_pool(name="pd_dr", bufs=3, space="DRAM") as pd_dr:
            for t in range(pred_steps):
                # L0: whh matmuls first (ready), then x-dependent ih
                psA = pd_ps.tile([BC, 512], F32, tag="A")
                psB = pd_ps.tile([BC, 256], F32, tag="B")
                psC = pd_ps.tile([BC, 256], F32, tag="C")
                for k in range(8):
                    nc.tensor.matmul(psA[:], h0T[:, k * 128:(k + 1) * 128], whh0_k[k][:, 0:512],
                                     start=(k == 0), stop=False)
                for k in range(8):
                    nc.tensor.matmul(psB[:], h0T[:, k * 128:(k + 1) * 128], whh0_k[k][:, 512:768],
                                     start=(k == 0), stop=False)
                nc.tensor.matmul(psB[:], ones_col[:, 0:BC], bnh0_sb[:], start=False, stop=True)
                psA1 = pd_ps2.tile([BC, 512], F32, tag="P")
                psB1 = pd_ps.tile([BC, 256], F32, tag="B1")
                psC1 = pd_ps.tile([BC, 256], F32, tag="C1")
                for k in range(8):
                    nc.tensor.matmul(psA1[:], h1T[:, k * 128:(k + 1) * 128], whh1_k[k][:, 0:512],
                                     start=(k == 0), stop=False)
                for k in range(8):
                    nc.tensor.matmul(psB1[:], h1T[:, k * 128:(k + 1) * 128], whh1_k[k][:, 512:768],
                                     start=(k == 0), stop=False)
                nc.tensor.matmul(psB1[:], ones_col[:, 0:BC], bnh1_sb[:], start=False, stop=True)
                if t == 0:
                    ihh, ihl = x0Th_sb[:], x0Tl_sb[:]
                else:
                    pT = pd_tr.tile([128, 128], BF16, tag="Tr")
                    nc.tensor.transpose(pT[:], xnew[:, 0:128], ident[:])
                    nc.vector.tensor_copy(xTh[:], pT[:])
                    pT2 = pd_tr.tile([8, 128], BF16, tag="Tr2")
                    nc.tensor.transpose(pT2[:], xnew[:, 128:136], ident[:])
                    nc.vector.tensor_copy(xTl[:], pT2[:])
                    ihh, ihl = xTh[:], xTl[:]
                nc.tensor.matmul(psA[:], ihh, wih0hi[:, 0:512], start=False, stop=False)
                nc.tensor.matmul(psA[:], ihl, wih0lo_sb[:, 0:512], start=False, stop=True)
                nc.tensor.matmul(psC[:], ihh, wih0hi[:, 512:768], start=True, stop=False)
                nc.tensor.matmul(psC[:], ihl, wih0lo_sb[:, 512:768], start=False, stop=True)
                rz = pd_sb.tile([BC, 512], F32, tag="rz")
                nc.scalar.activation(rz[:], psA[:], AF.Sigmoid)
                t1 = pd_sb.tile([BC, 256], F32, tag="t1")
                nc.vector.tensor_tensor(out=t1[:], in0=psB[:], in1=rz[:, 0:256], op=ALU.mult)
                nc.vector.tensor_tensor(out=t1[:], in0=t1[:], in1=psC[:], op=ALU.add)
                nn_ = pd_sb.tile([BC, 256], F32, tag="nn")
                nc.scalar.activation(nn_[:], t1[:], AF.Tanh)
                nc.vector.tensor_tensor(out=t1[:], in0=h0[:], in1=nn_[:], op=ALU.subtract)
                nc.vector.tensor_tensor(out=t1[:], in0=t1[:], in1=rz[:, 256:512], op=ALU.mult)
                nc.vector.tensor_tensor(out=h0[:], in0=nn_[:], in1=t1[:], op=ALU.add)
                hb = pd_sb.tile([BC, 256], BF16, tag="hb0")
                nc.vector.tensor_copy(hb[:], h0[:])
                agin0 = pd_sb.tile([128, 256], BF16, tag="agin0")
                for half in range(2):
                    pT = pd_tr.tile([128, 128], BF16, tag="Tr")
                    nc.tensor.transpose(pT[:], hb[:, half * 128:(half + 1) * 128], ident[:])
                    nc.vector.tensor_copy(agin0[:, half * 128:(half + 1) * 128], pT[:])
                emit_ag(pd_dr, 0, agin0, h0T, nc.sync, nc.sync)
                agin1 = emit_cell(pd_ps, pd_tr, pd_sb, pd_dr, 1, t,
                                  whh1_k, brz1_sb, bnh1_sb, bni1_sb, h1, h1T, ih1_fn,
                                  pre_mm=(psA1, psB1, psC1))
                emit_ag(pd_dr, 1, agin1, h1T, nc.sync, nc.gpsimd)
                # pre + ctx + relu -> hidden
                hid = pd_sb.tile([BC, H], F32, tag="hid")
                for s in range(2):
                    sl = slice(s * 512, (s + 1) * 512)
                    pP = pd_ps2.tile([BC, 512], F32, tag="P")
                    for k in range(8):
                        nc.tensor.matmul(pP[:], h1T[:, k * 128:(k + 1) * 128], pre_k[k][:, sl],
                                         start=(k == 0), stop=False)
                    nc.tensor.matmul(pP[:], ones_col[:, 0:BC], pre_b_sb[:, sl], start=False, stop=True)
                    nc.scalar.activation(hid[:, sl], pP[:], AF.Relu)
                nc.vector.tensor_tensor(out=hid[:], in0=hid[:], in1=ctx_sb[:], op=ALU.add)
                hidb = pd_sb.tile([BC, H], BF16, tag="hidb")
                nc.vector.tensor_copy(hidb[:], hid[:])
                hidT = pd_sb.tile([128, H], BF16, tag="hidT")
                for k in range(8):
                    pT = pd_tr.tile([128, 128], BF16, tag="Tr")
                    nc.tensor.transpose(pT[:], hidb[:, k * 128:(k + 1) * 128], ident[:])
                    nc.vector.tensor_copy(hidT[:, k * 128:(k + 1) * 128], pT[:])
                # spline
                z1 = pd_sb.tile([BC, JC * 128], BF16, tag="z1")
                pZ = pd_ps2.tile([BC, 512], F32, tag="P")
                for k in range(8):
                    nc.tensor.matmul(pZ[:], hidT[:, k * 128:(k + 1) * 128], spl1_k[k][:],
                                     start=(k == 0), stop=False)
                nc.tensor.matmul(pZ[:], ones_col[:, 0:BC], spl1b_sb[:], start=False, stop=True)
                nc.scalar.activation(z1[:], pZ[:], AF.Relu)
                pD = pd_ps2.tile([BC, JC * 6], F32, tag="P")
                for j in range(JC):
                    pT = pd_tr.tile([128, 128], BF16, tag="Tr")
                    nc.tensor.transpose(pT[:], z1[:, j * 128:(j + 1) * 128], ident[:])
                    z1T = pd_sb.tile([128, BC], BF16, tag="z1T")
                    nc.vector.tensor_copy(z1T[:], pT[:])
                    nc.tensor.matmul(pD[:, j * 6:(j + 1) * 6], z1T[:], spl2_sb[:, j * 6:(j + 1) * 6],
                                     start=True, stop=False)
                    nc.tensor.matmul(pD[:, j * 6:(j + 1) * 6], ones_col[:, 0:BC],
                                     spl2b_sb[:, j * 6:(j + 1) * 6], start=False, stop=True)
                nc.vector.tensor_tensor(out=p6d[:], in0=p6d[:], in1=pD[:], op=ALU.add)
                nc.sync.dma_start(out=out6d[t, :, :], in_=p6d[:])
                # rot6d -> R -> own x cols, AllGather, rebuild xnew
                v6 = p6d[:].rearrange("p (j r w) -> p j r w", r=3, w=2)
                a1, a2 = v6[:, :, :, 0], v6[:, :, :, 1]
                t12 = pd_sb.tile([BC, JC * 3], F32, tag="t12")
                t12v = t12[:].rearrange("p (j c) -> p j c", c=3)
                n4 = pd_sb.tile([BC, JC], F32, tag="n4")
                r4_ = pd_sb.tile([BC, JC], F32, tag="r4_")
                B1 = pd_sb.tile([BC, JC * 3], F32, tag="B1")
                B1v = B1[:].rearrange("p (j c) -> p j c", c=3)
                B2 = pd_sb.tile([BC, JC * 3], F32, tag="B2")
                B2v = B2[:].rearrange("p (j c) -> p j c", c=3)
                B3 = pd_sb.tile([BC, JC * 3], F32, tag="B3")
                B3v = B3[:].rearrange("p (j c) -> p j c", c=3)

                def normize(src, dst):
                    nc.vector.tensor_tensor(out=t12v, in0=src, in1=src, op=ALU.mult)
                    nc.vector.reduce_sum(n4[:, :, None], t12v, axis=mybir.AxisListType.X)
                    nc.scalar.activation(n4[:], n4[:], AF.Sqrt)
                    nc.vector.tensor_scalar_max(n4[:], n4[:], 1e-12)
                    nc.vector.reciprocal(r4_[:], n4[:])
                    nc.vector.tensor_tensor(out=dst, in0=src,
                                            in1=r4_[:, :, None].broadcast_to([BC, JC, 3]), op=ALU.mult)

                normize(a1, B1v)
                nc.vector.tensor_tensor(out=t12v, in0=B1v, in1=a2, op=ALU.mult)
                nc.vector.reduce_sum(n4[:, :, None], t12v, axis=mybir.AxisListType.X)
                nc.vector.tensor_tensor(out=t12v, in0=B1v,
                                        in1=n4[:, :, None].broadcast_to([BC, JC, 3]), op=ALU.mult)
                a2o = pd_sb.tile([BC, JC * 3], F32, tag="a2o")
                a2ov = a2o[:].rearrange("p (j c) -> p j c", c=3)
                nc.vector.tensor_tensor(out=a2ov, in0=a2, in1=t12v, op=ALU.subtract)
                normize(a2ov, B2v)
                for c in range(3):
                    u, v = (c + 1) % 3, (c + 2) % 3
                    m1 = pd_sb.tile([BC, JC], F32, tag="m1")
                    nc.vector.tensor_tensor(out=m1[:, :, None], in0=B1v[:, :, u:u + 1],
                                            in1=B2v[:, :, v:v + 1], op=ALU.mult)
                    m2 = pd_sb.tile([BC, JC], F32, tag="m2")
                    nc.vector.tensor_tensor(out=m2[:, :, None], in0=B1v[:, :, v:v + 1],
                                            in1=B2v[:, :, u:u + 1], op=ALU.mult)
                    nc.vector.tensor_tensor(out=B3v[:, :, c:c + 1], in0=m1[:, :, None],
                                            in1=m2[:, :, None], op=ALU.subtract)
                xn = pd_sb.tile([BC, JC * 9], BF16, tag="xn")
                xnv = xn[:].rearrange("p (j r c) -> p j r c", r=3, c=3)
                nc.vector.tensor_copy(xnv[:, :, :, 0:1], B1v[:, :, :, None])
                nc.vector.tensor_copy(xnv[:, :, :, 1:2], B2v[:, :, :, None])
                nc.vector.tensor_copy(xnv[:, :, :, 2:3], B3v[:, :, :, None])
                if t < pred_steps - 1:
                    xg_i = pd_dr.tile([BC, JC * 9], BF16, tag="xgi")
                    xg_o = pd_dr.tile([4 * BC, JC * 9], BF16, tag="xgo")
                    nc.sync.dma_start(out=xg_i[:], in_=xn[:])
                    nc.gpsimd.collective_compute("AllGather", ALU.bypass, ins=[xg_i.opt()],
                                                 outs=[xg_o.opt()], replica_groups=GROUPS)
                    nc.sync.dma_start(
                        out=xnew[:, 0:108].rearrange("p (r c) -> p r c", r=3),
                        in_=xg_o[0:384, :].rearrange("(r p) c -> p r c", r=3))
                    nc.sync.dma_start(out=xnew[:, 108:135], in_=xg_o[384:512, 0:27])
        roll_scope.__exit__(None, None, None)
        wp2_cm.__exit__(None, None, None)
        wp_cm.__exit__(None, None, None)
    nc.compile()
    return nc


# ---------------- host side ----------------
_cached = {}


class _SpmdRunner:
    def __init__(self, nc, n_cores):
        import jax
        from jax.sharding import Mesh, PartitionSpec
        from jax.experimental.shard_map import shard_map
        from concourse import bass2jax
        from concourse.bass2jax import _bass_exec_p, partition_id_tensor
        bass2jax.install_neuronx_cc_hook()
        self.jax = jax
        self.PartitionSpec = PartitionSpec
        self.n_cores = n_cores
        in_names, out_names, out_avals, zero_outs = [], [], [], []
        pname = nc.partition_id_tensor.name if nc.partition_id_tensor else None
        for alloc in nc.m.functions[0].allocations:
            if not isinstance(alloc, mybir.MemoryLocationSet):
                continue
            name = alloc.memorylocations[0].name
            if alloc.kind == "ExternalInput":
                if name != pname:
                    in_names.append(name)
            elif alloc.kind == "ExternalOutput":
                out_names.append(name)
                shape = tuple(alloc.tensor_shape)
                dtype = mybir.dt.np(alloc.dtype)
                out_avals.append(jax.core.ShapedArray(shape, dtype))
                zero_outs.append(np.zeros(shape, dtype))
        self.in_names, self.out_names = in_names, out_names
        self.out_avals, self.zero_outs = out_avals, zero_outs
        n_params, n_outs = len(in_names), len(out_names)
        all_in = in_names + out_names + ([pname] if pname else [])

        def _body(*args):
            operands = list(args)
            if pname is not None:
                operands.append(partition_id_tensor())
            return tuple(_bass_exec_p.bind(
                *operands, out_avals=tuple(out_avals), in_names=tuple(all_in),
                out_names=tuple(out_names), lowering_input_output_aliases=(),
                sim_require_finite=True, sim_require_nnan=True, nc=nc))

        devices = jax.devices()[:n_cores]
        self.mesh = Mesh(np.asarray(devices), ("core",))
        specs = (PartitionSpec("core"),) * (n_params + n_outs)
        self.fn = jax.jit(shard_map(_body, mesh=self.mesh, in_specs=specs,
                                    out_specs=(PartitionSpec("core"),) * n_outs,
                                    check_rep=False), keep_unused=True)

    def put(self, in_maps):
        import jax
        from jax.sharding import NamedSharding
        sh = NamedSharding(self.mesh, self.PartitionSpec("core"))
        args = []
        for name in self.in_names:
            arr = np.concatenate([np.asarray(m[name]) for m in in_maps], axis=0)
            args.append(jax.device_put(arr, sh))
        for z in self.zero_outs:
            args.append(jax.device_put(np.concatenate([z] * self.n_cores, axis=0), sh))
        return args

    def run(self, args):
        import jax
        outs = self.fn(*args)
        jax.block_until_ready(outs)
        return outs

    def results(self, outs):
        res = []
        for c in range(self.n_cores):
            d = {}
            for i, name in enumerate(self.out_names):
                d[name] = np.asarray(outs[i]).reshape(self.n_cores, *self.out_avals[i].shape)[c]
            res.append(d)
        return res


def get_runner(t_steps=T_STEPS, pred_steps=PRED_STEPS):
    key = (t_steps, pred_steps)
    if key not in _cached:
        nc = build_module(t_steps, pred_steps)
        _cached[key] = _SpmdRunner(nc, 8)
    return _cached[key]


def make_in_maps(inputs, t_steps=T_STEPS):
    poses = np.asarray(inputs["poses"], np.float32)
    freq_w, freq_b = inputs["freq_w"], inputs["freq_b"]
    attn_in_w, attn_in_b = inputs["attn_in_w"], inputs["attn_in_b"]
    attn_out_w, attn_out_b = inputs["attn_out_w"], inputs["attn_out_b"]
    Wf = (freq_w.T.astype(np.float64) @ attn_in_w.T.astype(np.float64)).astype(np.float32)
    bfull = (freq_b.astype(np.float64) @ attn_in_w.T.astype(np.float64)).astype(np.float32) + attn_in_b
    k_ = np.arange(FQ)[None, :]
    t_ = np.arange(T)[:, None]
    ct = np.cos(2 * np.pi * k_ * t_ / T).astype(np.float32)  # [T, FQ]

    x0 = poses[:, T - 1, :]
    R0 = x0.reshape(B, J, 3, 3)
    prev6d0 = np.concatenate([R0[..., 0], R0[..., 1]], axis=-1).reshape(B, J * 6)

    def hsl(w, l):  # w [.., 3072] -> gate slice cols for lane l
        r = w[..., l * 256:(l + 1) * 256]
        z = w[..., 1024 + l * 256:1024 + (l + 1) * 256]
        n = w[..., 2048 + l * 256:2048 + (l + 1) * 256]
        return np.concatenate([r, z, n], axis=-1)

    def bsl(b1, b2, l):
        s = b1 + b2
        return (np.concatenate([s[l * 256:(l + 1) * 256],
                                s[1024 + l * 256:1024 + (l + 1) * 256]])[None],
                b1[2048 + l * 256:2048 + (l + 1) * 256][None],
                b2[2048 + l * 256:2048 + (l + 1) * 256][None])

    in_maps = []
    for c in range(8):
        g, l = c // 4, c % 4
        bs = slice(g * BC, (g + 1) * BC)
        joints = [min(l * 4 + jj, J - 1) for jj in range(JC)]
        # attention lane slices (q|k|v own 256 cols each)
        wfh = np.concatenate([Wf[:, l * 256:(l + 1) * 256],
                              Wf[:, 1024 + l * 256:1024 + (l + 1) * 256],
                              Wf[:, 2048 + l * 256:2048 + (l + 1) * 256]], axis=1)
        bfh = np.concatenate([bfull[l * 256:(l + 1) * 256],
                              bfull[1024 + l * 256:1024 + (l + 1) * 256],
                              bfull[2048 + l * 256:2048 + (l + 1) * 256]])[None]
        wfq_lo = np.concatenate([wfh[128:D], bfh], axis=0)  # [8, 768]
        brz0_, bni0_, bnh0_ = bsl(inputs["gru_bih0"], inputs["gru_bhh0"], l)
        brz1_, bni1_, bnh1_ = bsl(inputs["gru_bih1"], inputs["gru_bhh1"], l)
        wih0 = hsl(inputs["gru_wih0"].T, l)                 # [135, 768]
        wih0lo_ = np.concatenate([wih0[128:D],
                                  np.concatenate([brz0_, bni0_], axis=1)], axis=0)
        # x^T layouts
        pg = poses[bs, :t_steps, :]                          # [BC, t, D]
        xt = pg.transpose(2, 1, 0)                           # [D, t, BC]
        XThi_ = xt[0:128].reshape(128, t_steps * 128)
        XTlo_ = np.concatenate([xt[128:D].reshape(7, t_steps * 128),
                                np.ones((1, t_steps * 128), np.float32)], axis=0)
        x0T_ = x0[bs].T                                      # [D, BC]
        x0Tl_ = np.concatenate([x0T_[128:D], np.ones((1, BC), np.float32)], axis=0)
        m = {
            "XThi": _bf(XThi_), "XTlo": _bf(XTlo_),
            "x0Th": _bf(x0T_[0:128]), "x0Tl": _bf(x0Tl_),
            "prev6d0_s": np.concatenate([prev6d0[bs, j * 6:(j + 1) * 6] for j in joints],
                                        axis=1).astype(np.float32),
            "posesT16": pg.transpose(1, 0, 2).reshape(t_steps, BC * D).astype(np.float16),
            "ct16": ct.astype(np.float16), "ctT_b": _bf(ct.T),
            "wfq_hi": wfh[0:128].astype(np.float16),
            "wfq_lo": wfq_lo.astype(np.float16),
            "wv_hi": _bf(wfh[0:128, 512:768]), "wv_lo": _bf(wfq_lo[:, 512:768]),
            "wout_h": _bf(attn_out_w[:, l * 256:(l + 1) * 256].T),
            "aob4": _bf((attn_out_b / 4.0)[None]),
            "wih0T_s": _bf(wih0[0:128]), "wih0lo_s": _bf(wih0lo_),
            "whh0T_s": _bf(hsl(inputs["gru_whh0"].T, l)),
            "wih1T_s": _bf(hsl(inputs["gru_wih1"].T, l)),
            "whh1T_s": _bf(hsl(inputs["gru_whh1"].T, l)),
            "bnh0": _bf(bnh0_),
            "brz1": _bf(brz1_), "bni1": _bf(bni1_), "bnh1": _bf(bnh1_),
            "pre_wT": _bf(inputs["pre_w"].T), "pre_b_row": _bf(inputs["pre_b"][None]),
            "spl1T": _bf(np.concatenate([inputs["spl_w1"][j].T for j in joints], axis=1)),
            "spl1b": _bf(np.concatenate([inputs["spl_b1"][j] for j in joints])[None]),
            "spl2_s": _bf(np.concatenate([inputs["spl_w2"][j].T for j in joints], axis=1)),
            "spl2b": _bf(np.concatenate([inputs["spl_b2"][j] for j in joints])[None]),
        }
        in_maps.append(m)
    return in_maps


def assemble_output(res, pred_steps=PRED_STEPS):
    pred6d = np.zeros((B, pred_steps, J * 6), np.float32)
    for c in range(8):
        g, l = c // 4, c % 4
        o = res[c]["out6d"][:pred_steps]
        njc = 4 if l < 3 else 3
        for jj in range(njc):
            jg = l * 4 + jj
            pred6d[g * BC:(g + 1) * BC, :, jg * 6:(jg + 1) * 6] = \
                o[:, :, jj * 6:(jj + 1) * 6].transpose(1, 0, 2)
    return pred6d


def kernel(**inputs):
    runner = get_runner()
    in_maps = make_in_maps(inputs)
    args = runner.put(in_maps)
    res = runner.results(runner.run(args))
    return assemble_output(res)
